# revision 21
# baseline (speedup 1.0000x reference)
"""Trainium2 Bass kernel for nn_Attention_25769804179.

Multi-head attention (B=4, S=2048, D=1024, H=16, hd=64), fp32 I/O.

Sharding: batch (4-way) x head-group (2-way, 8 heads each) over 8 NeuronCores.
Each core computes, for its batch b and head group g:
  qkv projection (its heads only), per-head softmax(q k^T / 8) v in a
  transposed-scores layout, and a partial output projection
  ctx @ W_proj[rows of g].  The host sums the two partials per batch and adds
  the bias terms.

Differences vs the fp32r baseline:
  - all tensor data is fp16 (x, weights, q/k/v, exp(scores), ctx); matmul
    accumulation stays fp32 in PSUM, so accuracy is ~1e-3 while every matmul
    runs at 1 cycle/row and FWL (fast weight load) stays enabled.
  - x is loaded to SBUF once (baseline re-streamed all of x per head-pair:
    32MB of DMA -> 4MB).
  - all four pairs' q^T/k^T tiles stay resident in SBUF (fp16 makes it fit),
    so stage 1 runs as one dense block the scheduler can overlap with stage 2.
  - the two K=64 score matmuls of each k-tile (one per head of the pair) are
    issued with explicit tile_position (0,0)/(64,0) so they run concurrently
    in disjoint row-groups of the PE array.
  - normalization reads PV results straight from PSUM (no staging copies):
    reciprocal of the Z row, a K=1 f32r broadcast matmul, and one
    tensor_tensor multiply writing fp16 ctx.
"""

import math
import sys
import time

sys.path.insert(0, "/opt/trn_rl_repo")

import numpy as np
import ml_dtypes

B, S, D = 4, 2048, 1024
NH, HD = 16, 64
HPC = 8          # heads per core
NPAIR = HPC // 2
SCALE = HD ** -0.5
NKT = S // 128   # 16 k-tiles
NSQ = S // 512   # 4 q-tiles of 512
NDT = D // 128   # 8 d-tiles
NCORES = 8

_RUNNER = None


def _build(repeat=1):
    import concourse.mybir as mybir
    import concourse.tile as tile
    from concourse import bacc

    dt = mybir.dt
    f32, f32r, f16 = dt.float32, dt.float32r, dt.float16
    AF = mybir.ActivationFunctionType
    ALU = mybir.AluOpType

    nc = bacc.Bacc("TRN2", debug=False, enable_partition_id=False)

    xt_d = nc.dram_tensor("xt", [NSQ, 128, NDT * 512], f16, kind="ExternalInput").ap()
    wqk_d = nc.dram_tensor("wqk", [2, NPAIR, 128, NDT * 128], f16, kind="ExternalInput").ap()
    wv_d = nc.dram_tensor("wv", [128, NDT * 512], f16, kind="ExternalInput").ap()
    wp_d = nc.dram_tensor("wp", [128, 4 * 1024], f16, kind="ExternalInput").ap()
    bqk_d = nc.dram_tensor("bqk", [128, 2 * NPAIR], f32, kind="ExternalInput").ap()
    out_d = nc.dram_tensor("out", [S // 128, 128, D], f16, kind="ExternalOutput").ap()

    with tile.TileContext(nc) as tc:
        with tc.tile_pool(name="consts", bufs=1) as consts, \
             tc.tile_pool(name="xts", bufs=1) as xt_pool, \
             tc.tile_pool(name="wqks", bufs=2) as wqk_pool, \
             tc.tile_pool(name="wvp", bufs=1) as wvp_pool, \
             tc.tile_pool(name="qkt", bufs=1) as qkt_pool, \
             tc.tile_pool(name="vat", bufs=2) as va_pool, \
             tc.tile_pool(name="ctx", bufs=1) as ctx_pool, \
             tc.tile_pool(name="ex", bufs=4) as ex_pool, \
             tc.tile_pool(name="rc", bufs=2) as rc_pool, \
             tc.tile_pool(name="osb", bufs=2) as out_pool, \
             tc.tile_pool(name="pssc", bufs=2, space="PSUM") as ps_sc, \
             tc.tile_pool(name="pspv", bufs=1, space="PSUM") as ps_pv, \
             tc.tile_pool(name="pss1", bufs=1, space="PSUM") as ps_s1, \
             tc.tile_pool(name="pstail", bufs=1, space="PSUM") as ps_tail:

            # ---------- persistent SBUF (outside the timed body) ----------
            bqk_sb = consts.tile([128, 2 * NPAIR], f32, name="bqk_sb")
            nc.sync.dma_start(bqk_sb[:], bqk_d[:])
            ones_f = consts.tile([1, 64], f32, name="ones_f")
            nc.vector.memset(ones_f[:], 1.0)
            ones_r = consts.tile([1, 64], f32r, name="ones_r")
            nc.vector.tensor_copy(ones_r[:], ones_f[:])

            def load_inputs(r):
                xt_sb = xt_pool.tile([128, NSQ * NDT * 512], f16, tag="xt", name=f"xt_{r}")
                for c in range(NSQ):
                    nc.sync.dma_start(xt_sb[:, c * 4096:(c + 1) * 4096], xt_d[c])
                wq_sb = wqk_pool.tile([128, NPAIR * 1024], f16, tag="wq", name=f"wq_{r}")
                wk_sb = wqk_pool.tile([128, NPAIR * 1024], f16, tag="wk", name=f"wk_{r}")
                for p in range(NPAIR):
                    nc.sync.dma_start(wq_sb[:, p * 1024:(p + 1) * 1024], wqk_d[0, p])
                    nc.sync.dma_start(wk_sb[:, p * 1024:(p + 1) * 1024], wqk_d[1, p])
                wv_sb = wvp_pool.tile([128, NDT * 512], f16, tag="wv", name=f"wv_{r}")
                nc.sync.dma_start(wv_sb[:], wv_d[:])
                wp_sb = wvp_pool.tile([128, 4 * 1024], f16, tag="wp", name=f"wp_{r}")
                nc.sync.dma_start(wp_sb[:], wp_d[:])
                return xt_sb, wq_sb, wk_sb, wv_sb, wp_sb

            def stage1_chunk(r, p, c, qt, kt, vav, xt_sb, wq_sb, wk_sb, wv_sb):
                # q/k projection for (pair p, chunk c); v for all heads when p==0
                xm = xt_sb[:, c * 4096:(c + 1) * 4096]
                for qk, (w_sb, dst) in enumerate(((wq_sb, qt[p]), (wk_sb, kt[p]))):
                    ps = ps_s1.tile([128, 512], f32, tag="s1", name=f"s1_{r}_{p}_{c}_{qk}")
                    for dtl in range(NDT):
                        nc.tensor.matmul(
                            ps[:],
                            w_sb[:, p * 1024 + dtl * 128: p * 1024 + (dtl + 1) * 128],
                            xm[:, dtl * 512:(dtl + 1) * 512],
                            start=(dtl == 0), stop=(dtl == NDT - 1),
                        )
                    nc.vector.tensor_scalar_add(
                        dst[:, c * 512:(c + 1) * 512], ps[:],
                        bqk_sb[:, qk * NPAIR + p: qk * NPAIR + p + 1],
                    )
                if p == 0:
                    for ss in range(4):
                        ktile = c * 4 + ss
                        ps = ps_s1.tile([128, 512], f32, tag="s1", name=f"v_{r}_{c}_{ss}")
                        for dtl in range(NDT):
                            nc.tensor.matmul(
                                ps[:],
                                xm[:, dtl * 512 + ss * 128: dtl * 512 + (ss + 1) * 128],
                                wv_sb[:, dtl * 512:(dtl + 1) * 512],
                                start=(dtl == 0), stop=(dtl == NDT - 1),
                            )
                        nc.vector.tensor_copy(
                            vav[:, ktile * HPC:(ktile + 1) * HPC, 0:64],
                            ps[:].rearrange("p (h e) -> p h e", e=64),
                        )

            def stage2_pair(r, p, qt, kt, vav, ctxt, wp_sb, s1_args, tail=False):
                for j in range(NSQ):
                    pv_lo = ps_pv.tile([65, 512], f32, tag="pvlo", name=f"pvl_{r}_{p}_{j}")
                    pv_hi = ps_pv.tile([65, 512], f32, tag="pvhi", name=f"pvh_{r}_{p}_{j}")
                    q_sl = qt[p][:, j * 512:(j + 1) * 512]
                    exs = {}
                    LAG = 2
                    for i in range(NKT + LAG):
                        if i < NKT:
                            sc = ps_sc.tile([128, 1024], f32, tag="sc", name=f"sc_{r}_{p}_{j}_{i}")
                            nc.tensor.matmul(sc[:, 0:512], kt[p][0:64, i * 128:(i + 1) * 128],
                                             q_sl[0:64, :], tile_position=(0, 0))
                            nc.tensor.matmul(sc[:, 512:1024], kt[p][64:128, i * 128:(i + 1) * 128],
                                             q_sl[64:128, :], tile_position=(64, 0))
                            ex = ex_pool.tile([128, 1024], f16, tag="ex", name=f"ex_{r}_{p}_{j}_{i}")
                            nc.scalar.activation(ex[:], sc[:], AF.Exp, scale=SCALE)
                            exs[i] = ex
                        if i >= LAG:
                            il = i - LAG
                            exl = exs.pop(il)
                            nc.tensor.matmul(
                                pv_lo[:], vav[:, il * HPC + 2 * p, :], exl[:, 0:512],
                                start=(il == 0), stop=(il == NKT - 1),
                            )
                            nc.tensor.matmul(
                                pv_hi[:], vav[:, il * HPC + 2 * p + 1, :], exl[:, 512:1024],
                                start=(il == 0), stop=(il == NKT - 1),
                            )
                    for h, pv in ((0, pv_lo), (1, pv_hi)):
                        rc = rc_pool.tile([1, 512], f32r, tag="rc", name=f"rc_{r}_{p}_{j}_{h}")
                        with nc.allow_low_precision(reason="1/Z in f32r: 10-bit mantissa ample for softmax norm"):
                            nc.vector.reciprocal(rc[:], pv[64:65, :])
                        bc = ps_tail.tile([64, 512], f32, tag="tail", name=f"bc_{r}_{p}_{j}_{h}")
                        nc.tensor.matmul(bc[:], ones_r[:], rc[:])
                        bc_sb = rc_pool.tile([64, 512], f16, tag="bcs", name=f"bcs_{r}_{p}_{j}_{h}")
                        nc.vector.tensor_copy(bc_sb[:], bc[:])
                        nc.vector.tensor_tensor(
                            ctxt[h * 64:(h + 1) * 64, p * S + j * 512: p * S + (j + 1) * 512],
                            bc_sb[:], pv[0:64, :], ALU.mult)
                    if tail:
                        stage3_block(r, ctxt, wp_sb, range(4 * j, 4 * j + 4))
                    else:
                        stage1_chunk(r, p + 1, j, qt, kt, vav, *s1_args)

            def stage3_block(r, ctxt, wp_sb, ts_range):
                for t in ts_range:
                    o_t = out_pool.tile([128, 1024], f16, tag="o", name=f"o_{r}_{t}")
                    for ch in range(2):
                        ps = ps_tail.tile([128, 512], f32, tag="tail", name=f"s3_{r}_{t}_{ch}")
                        for ft in range(NPAIR):
                            nc.tensor.matmul(
                                ps[:],
                                ctxt[:, ft * S + t * 128: ft * S + (t + 1) * 128],
                                wp_sb[:, ft * 1024 + ch * 512: ft * 1024 + (ch + 1) * 512],
                                start=(ft == 0), stop=(ft == NPAIR - 1),
                            )
                        nc.vector.tensor_copy(o_t[:, ch * 512:(ch + 1) * 512], ps[:])
                    nc.sync.dma_start(out_d[t], o_t[:])

            for r in range(repeat):
                xt_sb, wq_sb, wk_sb, wv_sb, wp_sb = load_inputs(r)
                qt = [qkt_pool.tile([128, S], f16, tag=f"qt{p}", name=f"qt{p}_{r}")
                      for p in range(NPAIR)]
                kt = [qkt_pool.tile([128, S], f16, tag=f"kt{p}", name=f"kt{p}_{r}")
                      for p in range(NPAIR)]
                va = va_pool.tile([128, NKT * HPC * 65], f16, tag="va", name=f"va_{r}")
                vav = va[:].rearrange("p (k c) -> p k c", c=65)
                nc.vector.memset(vav[:, :, 64:65], 1.0)
                s1_args = (xt_sb, wq_sb, wk_sb, wv_sb)
                for c in range(NSQ):
                    stage1_chunk(r, 0, c, qt, kt, vav, *s1_args)
                ctxt = ctx_pool.tile([128, NPAIR * S], f16, tag="ctxt", name=f"ctxt_{r}")
                for p in range(NPAIR):
                    stage2_pair(r, p, qt, kt, vav, ctxt, wp_sb, s1_args,
                                tail=(p == NPAIR - 1))

    nc.compile()
    return nc


def _make_runner(nc):
    import jax
    import jax.core as jcore
    from jax.experimental.shard_map import shard_map
    from jax.sharding import Mesh, NamedSharding, PartitionSpec

    import concourse.mybir as mybir
    from concourse import bass2jax

    bass2jax.install_neuronx_cc_hook()

    in_names, out_names, out_avals, zero_outs = [], [], [], []
    for alloc in nc.m.functions[0].allocations:
        if not isinstance(alloc, mybir.MemoryLocationSet):
            continue
        name = alloc.memorylocations[0].name
        if alloc.kind == "ExternalInput":
            in_names.append(name)
        elif alloc.kind == "ExternalOutput":
            out_names.append(name)
            shape = tuple(alloc.tensor_shape)
            npdt = mybir.dt.np(alloc.dtype)
            out_avals.append(jcore.ShapedArray(shape, npdt))
            zero_outs.append(np.zeros(shape, npdt))
    n_params = len(in_names)
    all_names = tuple(in_names + out_names)

    def _body(*args):
        outs = bass2jax._bass_exec_p.bind(
            *args,
            out_avals=tuple(out_avals),
            in_names=all_names,
            out_names=tuple(out_names),
            lowering_input_output_aliases=(),
            sim_require_finite=True,
            sim_require_nnan=True,
            nc=nc,
        )
        return tuple(outs)

    devices = jax.devices()[:NCORES]
    mesh = Mesh(np.asarray(devices), ("core",))
    nio = n_params + len(out_names)
    sharded = jax.jit(
        shard_map(
            _body, mesh=mesh,
            in_specs=(PartitionSpec("core"),) * nio,
            out_specs=(PartitionSpec("core"),) * len(out_names),
            check_rep=False,
        ),
        keep_unused=True,
    )
    sh = NamedSharding(mesh, PartitionSpec("core"))

    def run(in_maps, reps=1):
        concat = [
            np.concatenate([np.asarray(in_maps[c][n]) for c in range(NCORES)], axis=0)
            for n in in_names
        ]
        concat += [np.concatenate([z] * NCORES, axis=0) for z in zero_outs]
        dev_in = [jax.device_put(a, sh) for a in concat]
        outs = sharded(*dev_in)
        jax.block_until_ready(outs)
        per_call = None
        if reps > 1:
            t0 = time.perf_counter()
            for _ in range(reps - 1):
                outs = sharded(*dev_in)
            jax.block_until_ready(outs)
            per_call = (time.perf_counter() - t0) / (reps - 1)
        results = []
        for c in range(NCORES):
            d = {}
            for i, n in enumerate(out_names):
                arr = np.asarray(outs[i])
                d[n] = arr.reshape((NCORES,) + out_avals[i].shape)[c]
            results.append(d)
        return results, per_call

    return run


def _get_runner():
    global _RUNNER
    if _RUNNER is None:
        from concourse import bass_utils

        nc = _build()

        def run(in_maps, reps=1):
            if reps > 1:
                return _make_runner(nc)(in_maps, reps=reps)
            res = bass_utils.run_bass_kernel_spmd(nc, in_maps, core_ids=list(range(NCORES)))
            return res.results, None

        _RUNNER = run
    return _RUNNER


def _prep_in_maps(x, W_qkv, b_qkv, W_proj):
    F16 = np.float16
    in_maps = []
    for core in range(NCORES):
        b, g = core // 2, core % 2
        xT = np.ascontiguousarray(x[b].T)  # [D, S]
        xt = xT.reshape(NDT, 128, NSQ, 512).transpose(2, 1, 0, 3).reshape(NSQ, 128, NDT * 512)
        wq = W_qkv[:, g * 512:(g + 1) * 512]
        wk = W_qkv[:, 1024 + g * 512:1024 + (g + 1) * 512]
        wv = W_qkv[:, 2048 + g * 512:2048 + (g + 1) * 512]
        wqk = np.stack([
            w.reshape(NDT, 128, NPAIR, 128).transpose(2, 1, 0, 3).reshape(NPAIR, 128, NDT * 128)
            for w in (wq, wk)
        ])
        wv_t = wv.reshape(NDT, 128, 512).transpose(1, 0, 2).reshape(128, NDT * 512)
        wp_t = W_proj[g * 512:(g + 1) * 512].reshape(NPAIR, 128, 1024).transpose(1, 0, 2).reshape(128, NPAIR * 1024)
        bq = b_qkv[g * 512:(g + 1) * 512].reshape(NPAIR, 128).T
        bk = b_qkv[1024 + g * 512:1024 + (g + 1) * 512].reshape(NPAIR, 128).T
        bqk = np.concatenate([bq, bk], axis=1)
        in_maps.append({
            "xt": np.ascontiguousarray(xt).astype(F16),
            "wqk": np.ascontiguousarray(wqk).astype(F16),
            "wv": np.ascontiguousarray(wv_t).astype(F16),
            "wp": np.ascontiguousarray(wp_t).astype(F16),
            "bqk": np.ascontiguousarray(bqk, np.float32),
        })
    return in_maps


def _assemble(results, b_qkv, W_proj, b_proj):
    const = (b_qkv[2048:3072].astype(np.float64) @ W_proj.astype(np.float64)).astype(np.float32) + b_proj
    parts = [results[c]["out"].reshape(S, D).astype(np.float32) for c in range(NCORES)]
    out = np.stack([parts[2 * b] + parts[2 * b + 1] + const for b in range(B)])
    return out.astype(np.float32)


def kernel(x, W_qkv, b_qkv, W_proj, b_proj, _reps=1):
    x = np.asarray(x, np.float32)
    W_qkv = np.asarray(W_qkv, np.float32)
    b_qkv = np.asarray(b_qkv, np.float32)
    W_proj = np.asarray(W_proj, np.float32)
    b_proj = np.asarray(b_proj, np.float32)
    assert x.shape == (B, S, D), x.shape
    run = _get_runner()
    in_maps = _prep_in_maps(x, W_qkv, b_qkv, W_proj)
    results, per_call = run(in_maps, reps=_reps)
    kernel.last_per_call = per_call
    return _assemble(results, b_qkv, W_proj, b_proj)


kernel.last_per_call = None


# revision 22
# speedup vs baseline: 1.0944x; 1.0944x over previous
"""Trainium2 Bass kernel for nn_Attention_25769804179.

Multi-head attention (B=4, S=2048, D=1024, H=16, hd=64), fp32 I/O.

Sharding: batch (4-way) x head-group (2-way, 8 heads each) over 8 NeuronCores.
Each core computes, for its batch b and head group g:
  qkv projection (its heads only), per-head softmax(q k^T / 8) v in a
  transposed-scores layout, and a partial output projection
  ctx @ W_proj[rows of g].  The host sums the two partials per batch and adds
  the bias terms.

Differences vs the fp32r baseline:
  - all tensor data is fp16 (x, weights, q/k/v, exp(scores), ctx); matmul
    accumulation stays fp32 in PSUM, so accuracy is ~1e-3 while every matmul
    runs at 1 cycle/row and FWL (fast weight load) stays enabled.
  - x is loaded to SBUF once (baseline re-streamed all of x per head-pair:
    32MB of DMA -> 4MB).
  - all four pairs' q^T/k^T tiles stay resident in SBUF (fp16 makes it fit),
    so stage 1 runs as one dense block the scheduler can overlap with stage 2.
  - the two K=64 score matmuls of each k-tile (one per head of the pair) are
    issued with explicit tile_position (0,0)/(64,0) so they run concurrently
    in disjoint row-groups of the PE array.
  - normalization reads PV results straight from PSUM (no staging copies):
    reciprocal of the Z row, a K=1 f32r broadcast matmul, and one
    tensor_tensor multiply writing fp16 ctx.
"""

import math
import sys
import time

sys.path.insert(0, "/opt/trn_rl_repo")

import numpy as np
import ml_dtypes

B, S, D = 4, 2048, 1024
NH, HD = 16, 64
HPC = 8          # heads per core
NPAIR = HPC // 2
SCALE = HD ** -0.5
NKT = S // 128   # 16 k-tiles
NSQ = S // 512   # 4 q-tiles of 512
NDT = D // 128   # 8 d-tiles
NCORES = 8

_RUNNER = None


def _build(repeat=1):
    import concourse.mybir as mybir
    import concourse.tile as tile
    from concourse import bacc

    dt = mybir.dt
    f32, f32r, f16 = dt.float32, dt.float32r, dt.float16
    AF = mybir.ActivationFunctionType
    ALU = mybir.AluOpType

    nc = bacc.Bacc("TRN2", debug=False, enable_partition_id=False)

    xt_d = nc.dram_tensor("xt", [NSQ, 128, NDT * 512], f16, kind="ExternalInput").ap()
    wqk_d = nc.dram_tensor("wqk", [2, NPAIR, 128, NDT * 128], f16, kind="ExternalInput").ap()
    wv_d = nc.dram_tensor("wv", [128, NDT * 512], f16, kind="ExternalInput").ap()
    wp_d = nc.dram_tensor("wp", [128, 4 * 1024], f16, kind="ExternalInput").ap()
    bqk_d = nc.dram_tensor("bqk", [128, 2 * NPAIR], f32, kind="ExternalInput").ap()
    out_d = nc.dram_tensor("out", [S // 128, 128, D], f16, kind="ExternalOutput").ap()

    with tile.TileContext(nc) as tc:
        with tc.tile_pool(name="consts", bufs=1) as consts, \
             tc.tile_pool(name="xts", bufs=1) as xt_pool, \
             tc.tile_pool(name="wqks", bufs=2) as wqk_pool, \
             tc.tile_pool(name="wvp", bufs=1) as wvp_pool, \
             tc.tile_pool(name="qkt", bufs=1) as qkt_pool, \
             tc.tile_pool(name="vat", bufs=2) as va_pool, \
             tc.tile_pool(name="ctx", bufs=1) as ctx_pool, \
             tc.tile_pool(name="ex", bufs=4) as ex_pool, \
             tc.tile_pool(name="rc", bufs=2) as rc_pool, \
             tc.tile_pool(name="osb", bufs=2) as out_pool, \
             tc.tile_pool(name="pssc", bufs=2, space="PSUM") as ps_sc, \
             tc.tile_pool(name="pspv", bufs=1, space="PSUM") as ps_pv, \
             tc.tile_pool(name="pss1", bufs=1, space="PSUM") as ps_s1, \
             tc.tile_pool(name="pstail", bufs=1, space="PSUM") as ps_tail:

            # ---------- persistent SBUF (outside the timed body) ----------
            bqk_sb = consts.tile([128, 2 * NPAIR], f32, name="bqk_sb")
            nc.sync.dma_start(bqk_sb[:], bqk_d[:])
            ones_f = consts.tile([1, 64], f32, name="ones_f")
            nc.vector.memset(ones_f[:], 1.0)
            ones_r = consts.tile([1, 64], f32r, name="ones_r")
            nc.vector.tensor_copy(ones_r[:], ones_f[:])

            def load_inputs(r):
                xt_sb = xt_pool.tile([128, NSQ * NDT * 512], f16, tag="xt", name=f"xt_{r}")
                for c in range(NSQ):
                    nc.sync.dma_start(xt_sb[:, c * 4096:(c + 1) * 4096], xt_d[c])
                wq_sb = wqk_pool.tile([128, NPAIR * 1024], f16, tag="wq", name=f"wq_{r}")
                wk_sb = wqk_pool.tile([128, NPAIR * 1024], f16, tag="wk", name=f"wk_{r}")
                for p in range(NPAIR):
                    nc.sync.dma_start(wq_sb[:, p * 1024:(p + 1) * 1024], wqk_d[0, p])
                    nc.sync.dma_start(wk_sb[:, p * 1024:(p + 1) * 1024], wqk_d[1, p])
                wv_sb = wvp_pool.tile([128, NDT * 512], f16, tag="wv", name=f"wv_{r}")
                nc.sync.dma_start(wv_sb[:], wv_d[:])
                wp_sb = wvp_pool.tile([128, 4 * 1024], f16, tag="wp", name=f"wp_{r}")
                nc.sync.dma_start(wp_sb[:], wp_d[:])
                return xt_sb, wq_sb, wk_sb, wv_sb, wp_sb

            def stage1_chunk(r, p, c, qt, kt, vav, xt_sb, wq_sb, wk_sb, wv_sb):
                # q/k projection for (pair p, chunk c); v for all heads when p==0
                xm = xt_sb[:, c * 4096:(c + 1) * 4096]
                for qk, (w_sb, dst) in enumerate(((wq_sb, qt[p]), (wk_sb, kt[p]))):
                    ps = ps_s1.tile([128, 512], f32, tag="s1", name=f"s1_{r}_{p}_{c}_{qk}")
                    for dtl in range(NDT):
                        nc.tensor.matmul(
                            ps[:],
                            w_sb[:, p * 1024 + dtl * 128: p * 1024 + (dtl + 1) * 128],
                            xm[:, dtl * 512:(dtl + 1) * 512],
                            start=(dtl == 0), stop=(dtl == NDT - 1),
                        )
                    nc.vector.tensor_scalar_add(
                        dst[:, c * 512:(c + 1) * 512], ps[:],
                        bqk_sb[:, qk * NPAIR + p: qk * NPAIR + p + 1],
                    )
                if p == 0:
                    for ss in range(4):
                        ktile = c * 4 + ss
                        ps = ps_s1.tile([128, 512], f32, tag="s1", name=f"v_{r}_{c}_{ss}")
                        for dtl in range(NDT):
                            nc.tensor.matmul(
                                ps[:],
                                xm[:, dtl * 512 + ss * 128: dtl * 512 + (ss + 1) * 128],
                                wv_sb[:, dtl * 512:(dtl + 1) * 512],
                                start=(dtl == 0), stop=(dtl == NDT - 1),
                            )
                        nc.vector.tensor_copy(
                            vav[:, ktile * HPC:(ktile + 1) * HPC, 0:64],
                            ps[:].rearrange("p (h e) -> p h e", e=64),
                        )

            def stage2_pair(r, p, qt, kt, vav, ctxt, wp_sb, s1_args, tail=False):
                for j in range(NSQ):
                    pv_lo = ps_pv.tile([65, 512], f32, tag="pvlo", name=f"pvl_{r}_{p}_{j}")
                    pv_hi = ps_pv.tile([65, 512], f32, tag="pvhi", name=f"pvh_{r}_{p}_{j}")
                    q_sl = qt[p][:, j * 512:(j + 1) * 512]
                    exs = {}
                    for i in range(NKT + 1):
                        if i < NKT:
                            sc = ps_sc.tile([128, 1024], f32, tag="sc", name=f"sc_{r}_{p}_{j}_{i}")
                            nc.tensor.matmul(sc[:, 0:512], kt[p][0:64, i * 128:(i + 1) * 128],
                                             q_sl[0:64, :], tile_position=(0, 0))
                            nc.tensor.matmul(sc[:, 512:1024], kt[p][64:128, i * 128:(i + 1) * 128],
                                             q_sl[64:128, :], tile_position=(64, 0))
                            ex = ex_pool.tile([128, 1024], f16, tag="ex", name=f"ex_{r}_{p}_{j}_{i}")
                            nc.scalar.activation(ex[:], sc[:], AF.Exp, scale=SCALE)
                            exs[i] = ex
                        if i > 0:
                            exl = exs.pop(i - 1)
                            nc.tensor.matmul(
                                pv_lo[:], vav[:, (i - 1) * HPC + 2 * p, :], exl[:, 0:512],
                                start=(i - 1 == 0), stop=(i - 1 == NKT - 1),
                            )
                            nc.tensor.matmul(
                                pv_hi[:], vav[:, (i - 1) * HPC + 2 * p + 1, :], exl[:, 512:1024],
                                start=(i - 1 == 0), stop=(i - 1 == NKT - 1),
                            )
                    for h, pv in ((0, pv_lo), (1, pv_hi)):
                        rc = rc_pool.tile([1, 512], f32r, tag="rc", name=f"rc_{r}_{p}_{j}_{h}")
                        with nc.allow_low_precision(reason="1/Z in f32r: 10-bit mantissa ample for softmax norm"):
                            nc.vector.reciprocal(rc[:], pv[64:65, :])
                        bc = ps_tail.tile([64, 512], f32, tag="tail", name=f"bc_{r}_{p}_{j}_{h}")
                        nc.tensor.matmul(bc[:], ones_r[:], rc[:])
                        bc_sb = rc_pool.tile([64, 512], f16, tag="bcs", name=f"bcs_{r}_{p}_{j}_{h}")
                        nc.vector.tensor_copy(bc_sb[:], bc[:])
                        nc.vector.tensor_tensor(
                            ctxt[h * 64:(h + 1) * 64, p * S + j * 512: p * S + (j + 1) * 512],
                            bc_sb[:], pv[0:64, :], ALU.mult)
                    if tail:
                        stage3_block(r, ctxt, wp_sb, range(4 * j, 4 * j + 4))
                    else:
                        stage1_chunk(r, p + 1, j, qt, kt, vav, *s1_args)

            def stage3_block(r, ctxt, wp_sb, ts_range):
                for t in ts_range:
                    o_t = out_pool.tile([128, 1024], f16, tag="o", name=f"o_{r}_{t}")
                    for ch in range(2):
                        ps = ps_tail.tile([128, 512], f32, tag="tail", name=f"s3_{r}_{t}_{ch}")
                        for ft in range(NPAIR):
                            nc.tensor.matmul(
                                ps[:],
                                ctxt[:, ft * S + t * 128: ft * S + (t + 1) * 128],
                                wp_sb[:, ft * 1024 + ch * 512: ft * 1024 + (ch + 1) * 512],
                                start=(ft == 0), stop=(ft == NPAIR - 1),
                            )
                        nc.vector.tensor_copy(o_t[:, ch * 512:(ch + 1) * 512], ps[:])
                    nc.sync.dma_start(out_d[t], o_t[:])

            for r in range(repeat):
                xt_sb, wq_sb, wk_sb, wv_sb, wp_sb = load_inputs(r)
                qt = [qkt_pool.tile([128, S], f16, tag=f"qt{p}", name=f"qt{p}_{r}")
                      for p in range(NPAIR)]
                kt = [qkt_pool.tile([128, S], f16, tag=f"kt{p}", name=f"kt{p}_{r}")
                      for p in range(NPAIR)]
                va = va_pool.tile([128, NKT * HPC * 65], f16, tag="va", name=f"va_{r}")
                vav = va[:].rearrange("p (k c) -> p k c", c=65)
                nc.vector.memset(vav[:, :, 64:65], 1.0)
                s1_args = (xt_sb, wq_sb, wk_sb, wv_sb)
                for c in range(NSQ):
                    stage1_chunk(r, 0, c, qt, kt, vav, *s1_args)
                ctxt = ctx_pool.tile([128, NPAIR * S], f16, tag="ctxt", name=f"ctxt_{r}")
                for p in range(NPAIR):
                    stage2_pair(r, p, qt, kt, vav, ctxt, wp_sb, s1_args,
                                tail=(p == NPAIR - 1))

    nc.compile()
    return nc


def _make_runner(nc):
    import jax
    import jax.core as jcore
    from jax.experimental.shard_map import shard_map
    from jax.sharding import Mesh, NamedSharding, PartitionSpec

    import concourse.mybir as mybir
    from concourse import bass2jax

    bass2jax.install_neuronx_cc_hook()

    in_names, out_names, out_avals, zero_outs = [], [], [], []
    for alloc in nc.m.functions[0].allocations:
        if not isinstance(alloc, mybir.MemoryLocationSet):
            continue
        name = alloc.memorylocations[0].name
        if alloc.kind == "ExternalInput":
            in_names.append(name)
        elif alloc.kind == "ExternalOutput":
            out_names.append(name)
            shape = tuple(alloc.tensor_shape)
            npdt = mybir.dt.np(alloc.dtype)
            out_avals.append(jcore.ShapedArray(shape, npdt))
            zero_outs.append(np.zeros(shape, npdt))
    n_params = len(in_names)
    all_names = tuple(in_names + out_names)

    def _body(*args):
        outs = bass2jax._bass_exec_p.bind(
            *args,
            out_avals=tuple(out_avals),
            in_names=all_names,
            out_names=tuple(out_names),
            lowering_input_output_aliases=(),
            sim_require_finite=True,
            sim_require_nnan=True,
            nc=nc,
        )
        return tuple(outs)

    devices = jax.devices()[:NCORES]
    mesh = Mesh(np.asarray(devices), ("core",))
    nio = n_params + len(out_names)
    sharded = jax.jit(
        shard_map(
            _body, mesh=mesh,
            in_specs=(PartitionSpec("core"),) * nio,
            out_specs=(PartitionSpec("core"),) * len(out_names),
            check_rep=False,
        ),
        keep_unused=True,
    )
    sh = NamedSharding(mesh, PartitionSpec("core"))

    def run(in_maps, reps=1):
        concat = [
            np.concatenate([np.asarray(in_maps[c][n]) for c in range(NCORES)], axis=0)
            for n in in_names
        ]
        concat += [np.concatenate([z] * NCORES, axis=0) for z in zero_outs]
        dev_in = [jax.device_put(a, sh) for a in concat]
        outs = sharded(*dev_in)
        jax.block_until_ready(outs)
        per_call = None
        if reps > 1:
            t0 = time.perf_counter()
            for _ in range(reps - 1):
                outs = sharded(*dev_in)
            jax.block_until_ready(outs)
            per_call = (time.perf_counter() - t0) / (reps - 1)
        results = []
        for c in range(NCORES):
            d = {}
            for i, n in enumerate(out_names):
                arr = np.asarray(outs[i])
                d[n] = arr.reshape((NCORES,) + out_avals[i].shape)[c]
            results.append(d)
        return results, per_call

    return run


def _get_runner():
    global _RUNNER
    if _RUNNER is None:
        from concourse import bass_utils

        nc = _build()

        def run(in_maps, reps=1):
            if reps > 1:
                return _make_runner(nc)(in_maps, reps=reps)
            res = bass_utils.run_bass_kernel_spmd(nc, in_maps, core_ids=list(range(NCORES)))
            return res.results, None

        _RUNNER = run
    return _RUNNER


def _prep_in_maps(x, W_qkv, b_qkv, W_proj):
    F16 = np.float16
    in_maps = []
    for core in range(NCORES):
        b, g = core // 2, core % 2
        xT = np.ascontiguousarray(x[b].T)  # [D, S]
        xt = xT.reshape(NDT, 128, NSQ, 512).transpose(2, 1, 0, 3).reshape(NSQ, 128, NDT * 512)
        wq = W_qkv[:, g * 512:(g + 1) * 512]
        wk = W_qkv[:, 1024 + g * 512:1024 + (g + 1) * 512]
        wv = W_qkv[:, 2048 + g * 512:2048 + (g + 1) * 512]
        wqk = np.stack([
            w.reshape(NDT, 128, NPAIR, 128).transpose(2, 1, 0, 3).reshape(NPAIR, 128, NDT * 128)
            for w in (wq, wk)
        ])
        wv_t = wv.reshape(NDT, 128, 512).transpose(1, 0, 2).reshape(128, NDT * 512)
        wp_t = W_proj[g * 512:(g + 1) * 512].reshape(NPAIR, 128, 1024).transpose(1, 0, 2).reshape(128, NPAIR * 1024)
        bq = b_qkv[g * 512:(g + 1) * 512].reshape(NPAIR, 128).T
        bk = b_qkv[1024 + g * 512:1024 + (g + 1) * 512].reshape(NPAIR, 128).T
        bqk = np.concatenate([bq, bk], axis=1)
        in_maps.append({
            "xt": np.ascontiguousarray(xt).astype(F16),
            "wqk": np.ascontiguousarray(wqk).astype(F16),
            "wv": np.ascontiguousarray(wv_t).astype(F16),
            "wp": np.ascontiguousarray(wp_t).astype(F16),
            "bqk": np.ascontiguousarray(bqk, np.float32),
        })
    return in_maps


def _assemble(results, b_qkv, W_proj, b_proj):
    const = (b_qkv[2048:3072].astype(np.float64) @ W_proj.astype(np.float64)).astype(np.float32) + b_proj
    parts = [results[c]["out"].reshape(S, D).astype(np.float32) for c in range(NCORES)]
    out = np.stack([parts[2 * b] + parts[2 * b + 1] + const for b in range(B)])
    return out.astype(np.float32)


def kernel(x, W_qkv, b_qkv, W_proj, b_proj, _reps=1):
    x = np.asarray(x, np.float32)
    W_qkv = np.asarray(W_qkv, np.float32)
    b_qkv = np.asarray(b_qkv, np.float32)
    W_proj = np.asarray(W_proj, np.float32)
    b_proj = np.asarray(b_proj, np.float32)
    assert x.shape == (B, S, D), x.shape
    run = _get_runner()
    in_maps = _prep_in_maps(x, W_qkv, b_qkv, W_proj)
    results, per_call = run(in_maps, reps=_reps)
    kernel.last_per_call = per_call
    return _assemble(results, b_qkv, W_proj, b_proj)


kernel.last_per_call = None
